# revision 1
# baseline (speedup 1.0000x reference)
"""Trainium2 Bass kernel for nn_DeepTransitionRNN_31928786878509.

kernel(**inputs) -> np.ndarray, matching reference.reference semantics:
a deep-transition GRU over T=512 steps, B=128 (packed-sequence masking),
D=H=256, L=4 transition layers.

Strategy: data-parallel over batch (16 rows/core on 8 cores). Each core runs
the full 512-step recurrence on its shard with the hidden state resident in
SBUF in transposed layout h^T [H-chunk partitions x batch], weights stationary
as fp16 [128,128] chunks (fast FWL weight loads), PSUM fp32 accumulation,
fp32 elementwise, sigmoid/tanh on the scalar engine (shared act table).
Packed-sequence masking (out=0 for t >= lengths[b]) is applied on the host;
inactive rows' hidden states are allowed to free-run on device (row-
independent recurrence, lengths sorted descending, so this is exact).
"""

import os
import numpy as np
from contextlib import ExitStack

import concourse.bass as bass
import concourse.bacc as bacc
import concourse.mybir as mybir
import concourse.tile as tile
from concourse.bass_utils import run_bass_kernel_spmd

f16 = mybir.dt.float16
f32 = mybir.dt.float32
AF = mybir.ActivationFunctionType
OP = mybir.AluOpType

T, B, D, H, L = 512, 128, 256, 256, 4
NCORE = 8
BS = B // NCORE
KC_D = D // 128
KC_H = H // 128
MC = H // 128
NCH = 3 * 4 * MC + 3 * KC_H * MC + 3 * L * KC_H * MC  # 84

UNROLL = 8

LAST_EXEC_NS = None  # set by kernel() when tracing is enabled


def _pack_weights(Wr, Wz, Wl, Wt, Cx, Ch, Tr, Tz, Tn):
    chunks = []

    def add(M):
        for kc in range(M.shape[0] // 128):
            for mc in range(MC):
                chunks.append(M[kc * 128:(kc + 1) * 128, mc * 128:(mc + 1) * 128])

    add(Wr); add(Wz); add(Wl); add(Cx); add(Wt); add(Ch)
    for i in range(L):
        add(Tr[i]); add(Tz[i]); add(Tn[i])
    arr = np.stack([np.asarray(c, dtype=np.float32) for c in chunks])
    arr = arr.transpose(1, 0, 2).astype(np.float16)
    ident = np.eye(128, dtype=np.float16)[:, None, :]
    arr = np.ascontiguousarray(np.concatenate([arr, ident], axis=1))
    return arr


def _pack_x_shard(x_shard):
    Tn = x_shard.shape[0]
    y = np.asarray(x_shard, dtype=np.float16).reshape(Tn, BS, KC_D, 128)
    y = y.transpose(0, 3, 2, 1)
    return np.ascontiguousarray(y.reshape(Tn, 128, KC_D * BS))


def _build_nc(Tsteps, unroll):
    assert Tsteps % unroll == 0
    nc = bacc.Bacc(None, target_bir_lowering=False, debug=False)
    xin = nc.dram_tensor('xt', [Tsteps, 128, KC_D * BS], f16, kind='ExternalInput')
    win = nc.dram_tensor('wp', [128, NCH + 1, 128], f16, kind='ExternalInput')
    oul = nc.dram_tensor('out', [Tsteps, BS, H], f16, kind='ExternalOutput')

    idx = {}
    pos = 0

    def reg(name, kt):
        nonlocal pos
        idx[name] = [[pos + kc * MC + mc for mc in range(MC)] for kc in range(kt)]
        pos += kt * MC

    reg('Wr', 4); reg('Wz', 4); reg('Wl', 4)
    reg('Cx', 2); reg('Wt', 2); reg('Ch', 2)
    for i in range(L):
        reg(f'Tr{i}', 2); reg(f'Tz{i}', 2); reg(f'Tn{i}', 2)
    assert pos == NCH
    ID_CHUNK = NCH

    with ExitStack() as ctx:
        tc = ctx.enter_context(tile.TileContext(nc))
        wpool = ctx.enter_context(tc.tile_pool(name='w', bufs=1))
        hpool = ctx.enter_context(tc.tile_pool(name='h', bufs=1))
        spool = ctx.enter_context(tc.tile_pool(name='s', bufs=2))
        xpool = ctx.enter_context(tc.tile_pool(name='x', bufs=2))
        opool = ctx.enter_context(tc.tile_pool(name='o', bufs=2))
        ps_r_pool = ctx.enter_context(tc.tile_pool(name='ps_r', bufs=1, space='PSUM'))
        ps_zl_pool = ctx.enter_context(tc.tile_pool(name='ps_zl', bufs=1, space='PSUM'))
        ps_b_pool = ctx.enter_context(tc.tile_pool(name='ps_b', bufs=2, space='PSUM'))
        ps_rr_pool = ctx.enter_context(tc.tile_pool(name='ps_rr', bufs=1, space='PSUM'))
        ps_nz_pool = ctx.enter_context(tc.tile_pool(name='ps_nz', bufs=1, space='PSUM'))
        ps_o_pool = ctx.enter_context(tc.tile_pool(name='ps_o', bufs=2, space='PSUM'))

        W = wpool.tile([128, NCH + 1, 128], f16)
        nc.gpsimd.dma_start(W[:], win[:])

        # Every sub-layer blend (cell + 4 transition layers) is the same form
        #   h' = sig_gate * (h_or_d base diff) + base
        # and is computed by ONE fused tensor_tensor_scan over triplet-
        # interleaved [128, c, b, 3] tiles:
        #   d0 = W4 = [d, sig, 0],  d1 = W5 = [0, base, 1],  init = 1.0
        #   c0: s = d*s_prev + 0 = d  (s_prev = 1 from c2 of prior triplet)
        #   c1: s = sig*d + base = h' ; c2: s = 0*h' + 1 = 1  (re-arm)
        # so the output triplet is [d, h', 1] and j=1 carries h. h ping-pongs
        # between HTa/HTb.
        HTa = hpool.tile([128, KC_H, BS, 3], f16, tag='hta')
        HTb = hpool.tile([128, KC_H, BS, 3], f16, tag='htb')
        nc.gpsimd.memset(HTa[:], 0.0)
        nc.gpsimd.memset(HTb[:], 0.0)
        W4 = hpool.tile([128, KC_H, BS, 3], f32, tag='w4')
        W5 = hpool.tile([128, KC_H, BS, 3], f32, tag='w5')
        nc.gpsimd.memset(W4[:], 0.0)
        nc.gpsimd.memset(W5[:], 0.0)
        nc.gpsimd.memset(W5[:, :, :, 2], 1.0)

        def flat(ap):
            return ap.rearrange('p c b j -> p (c b j)')

        def mm(out_ap, name, kc, mc, rhs, start, stop):
            nc.tensor.matmul(out_ap, W[:, idx[name][kc][mc], :], rhs,
                             start=start, stop=stop)

        def emit_step(xt, ob_slice, hcur, hother):
            # h ping-pongs between hcur/hother per sub-layer; hseq[s] = tile
            # holding h BEFORE sub-layer s (s = 0 is the cell).
            hseq = [hcur if s % 2 == 0 else hother for s in range(2 + L)]

            def h_ap(tile_, kc):
                return tile_[:, kc, :, 1]

            ps_r = ps_r_pool.tile([128, MC, BS], f32, tag='ps_r')
            ps_zl = ps_zl_pool.tile([128, 2, MC, BS], f32, tag='ps_zl')
            ps_b = ps_b_pool.tile([128, 3, MC, BS], f32, tag='ps_b')

            for mc in range(MC):
                for kc in range(KC_D):
                    mm(ps_r[:, mc], 'Wr', kc, mc, xt[:, kc],
                       mc == 0 and kc == 0, False)
            # l before z so sig(l) is ready early (w_ feeds the tanh tail)
            for gi, g in ((1, 'Wl'), (0, 'Wz')):
                for mc in range(MC):
                    for kc in range(KC_D):
                        mm(ps_zl[:, gi, mc], g, kc, mc, xt[:, kc],
                           gi == 1 and mc == 0 and kc == 0, False)
            for mc in range(MC):
                for kc in range(KC_D):
                    mm(ps_b[:, 1, mc], 'Cx', kc, mc, xt[:, kc],
                       mc == 0 and kc == 0, False)
            for mc in range(MC):
                for kc in range(KC_D):
                    mm(ps_b[:, 2, mc], 'Wt', kc, mc, xt[:, kc], False, False)

            h0 = hseq[0]
            for mc in range(MC):
                for kc in range(KC_H):
                    mm(ps_r[:, mc], 'Wr', KC_D + kc, mc, h_ap(h0, kc), False,
                       mc == MC - 1 and kc == KC_H - 1)
            for mc in range(MC):
                for kc in range(KC_H):
                    mm(ps_b[:, 0, mc], 'Ch', kc, mc, h_ap(h0, kc), False,
                       mc == MC - 1 and kc == KC_H - 1)
            for gi, g in ((1, 'Wl'), (0, 'Wz')):
                for mc in range(MC):
                    for kc in range(KC_H):
                        mm(ps_zl[:, gi, mc], g, KC_D + kc, mc, h_ap(h0, kc), False,
                           gi == 0 and mc == MC - 1 and kc == KC_H - 1)

            s_r = spool.tile([128, MC, BS], f32, tag='s_r')
            nc.scalar.activation(s_r[:], ps_r[:], AF.Sigmoid)
            u = spool.tile([128, MC, BS], f32, tag='u')
            nc.vector.tensor_tensor(u[:], s_r[:], ps_b[:, 0], OP.mult)
            # v lands back in ps_r (r-preacts already consumed): ACT reads
            # PSUM 50 cycles faster than SBUF.
            nc.vector.tensor_tensor(ps_r[:], u[:], ps_b[:, 1], OP.add)
            s_l = spool.tile([128, MC, BS], f32, tag='s_l')
            nc.scalar.activation(s_l[:], ps_zl[:, 1], AF.Sigmoid)
            nnc = spool.tile([128, MC, BS], f32, tag='nnc')
            nc.scalar.activation(nnc[:], ps_r[:], AF.Tanh)
            # 1 - sig(zpre) = sig(-zpre): the cell blend becomes
            # h' = sig(-zpre)*(h - n) + n, same scan form as the layers.
            nc.scalar.activation(W4[:, :, :, 1], ps_zl[:, 0], AF.Sigmoid,
                                 scale=-1.0)
            w_ = spool.tile([128, MC, BS], f32, tag='w_')
            nc.vector.tensor_tensor(w_[:], s_l[:], ps_b[:, 2], OP.mult)
            nc.vector.tensor_tensor(W5[:, :, :, 1], nnc[:], w_[:], OP.add)
            nc.vector.tensor_tensor(W4[:, :, :, 0], h0[:, :, :, 1],
                                    W5[:, :, :, 1], OP.subtract)
            nc.vector.tensor_tensor_scan(
                flat(hseq[1][:]), flat(W4[:]), flat(W5[:]), 1.0,
                OP.mult, OP.add)

            for li in range(L):
                hp = hseq[1 + li]
                ps_rr = ps_rr_pool.tile([128, MC, BS], f32, tag='ps_rr')
                ps_nz = ps_nz_pool.tile([128, 2, MC, BS], f32, tag='ps_nz')
                for mc in range(MC):
                    for kc in range(KC_H):
                        mm(ps_rr[:, mc], f'Tr{li}', kc, mc, h_ap(hp, kc),
                           mc == 0 and kc == 0, mc == MC - 1 and kc == KC_H - 1)
                for mc in range(MC):
                    for kc in range(KC_H):
                        mm(ps_nz[:, 0, mc], f'Tn{li}', kc, mc, h_ap(hp, kc),
                           mc == 0 and kc == 0, False)
                for mc in range(MC):
                    for kc in range(KC_H):
                        mm(ps_nz[:, 1, mc], f'Tz{li}', kc, mc, h_ap(hp, kc), False,
                           mc == MC - 1 and kc == KC_H - 1)
                s_rr = spool.tile([128, MC, BS], f32, tag='s_rr')
                nc.scalar.activation(s_rr[:], ps_rr[:], AF.Sigmoid)
                # m lands back in ps_rr (rr-preacts consumed) for the faster
                # ACT PSUM read.
                nc.vector.tensor_tensor(ps_rr[:], s_rr[:], ps_nz[:, 0], OP.mult)
                nc.scalar.activation(W5[:, :, :, 1], ps_rr[:], AF.Tanh)
                nc.scalar.activation(W4[:, :, :, 1], ps_nz[:, 1], AF.Sigmoid)
                nc.vector.tensor_tensor(W4[:, :, :, 0], hp[:, :, :, 1],
                                        W5[:, :, :, 1], OP.subtract)
                # h' = sig(zz)*(h - nn) + nn
                nc.vector.tensor_tensor_scan(
                    flat(hseq[2 + li][:]), flat(W4[:]), flat(W5[:]), 1.0,
                    OP.mult, OP.add)

            hf = hseq[1 + L]
            ps_o = ps_o_pool.tile([BS, MC, 128], f16, tag='ps_o')
            for c in range(KC_H):
                nc.tensor.matmul(ps_o[:, c, :], h_ap(hf, c), W[:, ID_CHUNK, :],
                                 is_transpose=True, start=c == 0, stop=c == KC_H - 1)
            nc.vector.tensor_copy(ob_slice, ps_o[:].rearrange('p c f -> p (c f)'))
            return hf

        hcur, hother = HTa, HTb
        for tb in range(0, Tsteps, unroll):
            xt_tile = xpool.tile([128, unroll, KC_D * BS], f16, tag='xt')
            nc.sync.dma_start(
                xt_tile[:], xin[tb:tb + unroll].rearrange('u p f -> p u f'))
            ob_tile = opool.tile([BS, unroll, H], f16, tag='ob')
            for j in range(unroll):
                emit_step(xt_tile[:, j].rearrange('p (c b) -> p c b', c=KC_D),
                          ob_tile[:, j], hcur, hother)
                hcur, hother = hother, hcur
            nc.sync.dma_start(
                oul[tb:tb + unroll].rearrange('u b h -> b u h'), ob_tile[:])

    nc.compile()
    return nc


def _install_ntff_hook_shim():
    """The agent image lacks ``antenv.axon_hooks``; recreate it and register
    trn_boot's ctypes NTFF hook so trace=True works. Returns True on
    success."""
    import sys
    import types
    try:
        import antenv.axon_hooks  # noqa: F401
        return True
    except ImportError:
        pass
    try:
        import antenv
        from trn_agent_boot.trn_boot import _ntff_profile_via_ctypes
        mod = types.ModuleType('antenv.axon_hooks')
        mod._hook = _ntff_profile_via_ctypes('/opt/axon/libaxon_pjrt.so')
        mod.get_axon_ntff_profile_hook = lambda: mod._hook
        mod.set_axon_ntff_profile_hook = lambda h: setattr(mod, '_hook', h)
        sys.modules['antenv.axon_hooks'] = mod
        antenv.axon_hooks = mod
        return True
    except Exception as e:  # degrade to no-trace
        print(f'ntff hook shim failed: {e}')
        return False


def kernel(x, lengths, Wr, Wz, Wl, Wt, Cx, Ch, Tr, Tz, Tn):
    global LAST_EXEC_NS
    x = np.asarray(x)
    lengths = np.asarray(lengths)

    wp = _pack_weights(Wr, Wz, Wl, Wt, Cx, Ch, Tr, Tz, Tn)
    nc = _build_nc(T, UNROLL)

    in_maps = []
    for k in range(NCORE):
        xs = x[:, k * BS:(k + 1) * BS, :]
        in_maps.append({'xt': _pack_x_shard(xs), 'wp': wp})

    trace = bool(int(os.environ.get('RNN_KERNEL_TRACE', '0')))
    if trace:
        trace = _install_ntff_hook_shim()
    res = run_bass_kernel_spmd(nc, in_maps, core_ids=list(range(NCORE)),
                               trace=trace)
    LAST_EXEC_NS = res.exec_time_ns

    out = np.empty((T, B, H), np.float32)
    for k in range(NCORE):
        out[:, k * BS:(k + 1) * BS, :] = np.asarray(
            res.results[k]['out'], np.float32)
    mask = np.arange(T)[:, None] < lengths[None, :]
    out *= mask[:, :, None].astype(np.float32)
    return out

